# revision 45
# baseline (speedup 1.0000x reference)
"""Trainium2 Bass kernel for nn_AttentionBlock (B=8, L=2048, C=512, GroupNorm(8) +
single-head attention + residual), data-parallel over batch across 8 NeuronCores.

Self-contained: hardcodes shapes/sharding. kernel(**inputs) -> np.ndarray [B,L,C].

Dataflow (per core, one batch element, everything channel-major / "transposed"):
  x^T [C,L] (f32) --bn_stats/group-reduce--> h^T = a_c * x^T + b_c  (f32 + bf16 copy)
  Q^T = wq^T h^T + bq ;  K^T = (wk*scale)^T h^T + bk*scale  (scale folded on host)
  V   = h^T-chunks^T @ wv + bv           (natural [L,C] layout)
  per 512-wide lq tile:
     for each 128-key block: S^T = K^T-chunk^T @ Q^T (PSUM f32); P = exp(S^T) (bf16)
     O^T  += V-chunk^T @ P  (PSUM f32 accum over key blocks), denom += 1^T @ P
     out^T = h^T + (wp^T O^T) * (1/denom) + bp      (f32 combine)
Matmul operands are bf16 (1 cyc/row on PE); accumulation always fp32 in PSUM.
Host side transposes x per batch, casts weights to bf16, transposes output back.
"""

import numpy as np

B, L, C = 8, 2048, 512
GROUPS = 8
EPS = 1e-3
P = 128
CS = C // P            # 4 channel subtiles of 128
LQ = 512               # lq tile width (matmul free dim)
NLT = L // LQ          # 4 lq tiles
NLB = L // P           # 16 key/l blocks
CPG = C // GROUPS      # 64 channels per group
N_CORES = 8

_CACHE = {}


def _build_nc():
    from contextlib import ExitStack

    import concourse.bass as bass
    import concourse.mybir as mybir
    import concourse.tile as tile
    from concourse import bacc
    from concourse.bass import ts

    f32 = mybir.dt.float32
    bf16 = mybir.dt.bfloat16
    AF = mybir.ActivationFunctionType
    ALU = mybir.AluOpType

    nc = bacc.Bacc(trn_type="TRN2")

    xr_d = nc.dram_tensor("xr", [NLT, P, CS, LQ], f32, kind="ExternalInput")
    xb_d = nc.dram_tensor("xb", [C, L], bf16, kind="ExternalInput")
    w_d = {
        n: nc.dram_tensor(n, [P, CS, C], bf16, kind="ExternalInput")
        for n in ("wq", "wk", "wv", "wp")
    }
    # packed per-channel vectors: [gamma, beta, bq, bk, bp] x CS columns
    vp_d = nc.dram_tensor("vp", [P, 5 * CS], f32, kind="ExternalInput")
    bvb_d = nc.dram_tensor("bv_bcast", [P, C], f32, kind="ExternalInput")
    g0_d = nc.dram_tensor("g0", [P, 2], f32, kind="ExternalInput")
    sel_d = nc.dram_tensor("sel", [2, P], f32, kind="ExternalInput")
    out_d = nc.dram_tensor("out_t", [C, L], f32, kind="ExternalOutput")

    xb_dv = xb_d[:].rearrange("(s p) l -> p s l", p=P)
    out_dv = out_d[:].rearrange("(s p) l -> p s l", p=P)

    with tile.TileContext(nc) as tc, ExitStack() as ctx:
        consts = ctx.enter_context(tc.tile_pool(name="consts", bufs=1))
        data = ctx.enter_context(tc.tile_pool(name="data", bufs=1))
        small = ctx.enter_context(tc.tile_pool(name="small", bufs=1))
        ptp = ctx.enter_context(tc.tile_pool(name="ptp", bufs=3))
        oup = ctx.enter_context(tc.tile_pool(name="oup", bufs=4))
        finp = ctx.enter_context(tc.tile_pool(name="finp", bufs=2))
        psA = ctx.enter_context(tc.tile_pool(name="psA", bufs=4, space="PSUM"))
        psS = ctx.enter_context(tc.tile_pool(name="psS", bufs=3, space="PSUM"))
        psD = ctx.enter_context(tc.tile_pool(name="psD", bufs=1, space="PSUM"))

        # ---- SBUF residents ----
        xt = data.tile([P, CS, L], f32)       # x^T, then h^T (f32, residual)
        xb = data.tile([P, CS, L], bf16)      # x^T bf16, stats fast path
        hb = data.tile([P, CS, L], bf16)      # h^T bf16 (matmul operand)
        qt = data.tile([P, CS, L], bf16)      # Q^T
        kt = data.tile([P, CS, L], bf16)      # K^T (pre-scaled)
        vt = data.tile([P, NLB, C], bf16)     # V natural, [l%P, l//P, c]
        wsb = {n: consts.tile([P, CS, C], bf16, name=f"w_{n}") for n in w_d}
        vp = consts.tile([P, 5 * CS], f32)
        GAM, BET, BQ, BK, BP = (vp[:, i * CS:(i + 1) * CS] for i in range(5))
        bvb = consts.tile([P, C], f32)
        g0 = consts.tile([P, 2], f32)
        sel = consts.tile([2, P], f32)
        ones_col = consts.tile([P, 1], bf16)
        ones_row = consts.tile([1, P], bf16)
        eps2 = consts.tile([2, 1], f32)

        # ---- loads + constants ----
        # DMA order IS the critical path: tiny constants first (they gate the
        # stats reduce), then bf16 x (gates bn_stats), then weights (gate the
        # first projection matmuls), then f32 x (residual only — needed late).
        nc.gpsimd.dma_start(out=g0[:], in_=g0_d[:])
        nc.gpsimd.dma_start(out=sel[:], in_=sel_d[:])
        nc.gpsimd.dma_start(out=vp[:], in_=vp_d[:])
        for s in range(CS):
            nc.sync.dma_start(out=xb[:, s, :], in_=xb_dv[:, s, :])
        for n in ("wq", "wk", "wv", "wp"):
            nc.sync.dma_start(out=wsb[n][:], in_=w_d[n][:])
        nc.sync.dma_start(out=bvb[:], in_=bvb_d[:])
        # f32 x only feeds the residual add in the lt-th finale — stream it
        # per lq tile so it never competes with the latency-critical loads.
        for lt in range(NLT):
            nc.sync.dma_start(out=xt[:, :, ts(lt, LQ)], in_=xr_d[lt])
        nc.vector.memset(ones_col[:], 1.0)
        nc.vector.memset(ones_row[:], 1.0)
        nc.vector.memset(eps2[:], EPS)

        # ---- GroupNorm stats ----
        # per-channel (partition) mean/var over L via bn_stats, then group
        # aggregation across partitions with a tiny fp32 matmul.
        st = small.tile([P, CS, 2], f32)      # (mean_c, E[x^2]_c) per subtile
        # subtiles 0..2 on DVE (bn_stats); subtile 3 on the otherwise-idle ACT
        # via activation accum_out (sum and sum-of-squares along L).
        for s in range(CS - 1):
            st6 = small.tile([P, 4, 6], f32, tag="st6", bufs=2)
            for j in range(4):
                nc.vector.bn_stats(out=st6[:, j, :], in_=xb[:, s, ts(j, 512)])
            mv = small.tile([P, 2], f32, tag="mv", bufs=2)
            nc.vector.bn_aggr(out=mv[:], in_=st6[:])
            nc.vector.tensor_copy(out=st[:, s, 0:1], in_=mv[:, 0:1])
            nc.vector.tensor_tensor(out=st[:, s, 1:2], in0=mv[:, 0:1], in1=mv[:, 0:1], op=ALU.mult)
            nc.vector.tensor_tensor(out=st[:, s, 1:2], in0=st[:, s, 1:2], in1=mv[:, 1:2], op=ALU.add)
        s3 = CS - 1
        ssum = small.tile([P, 2], f32)        # (sum, sumsq) of subtile 3
        gscr = small.tile([P, L], bf16)
        nc.scalar.activation(out=gscr[:], in_=xb[:, s3, :], func=AF.Identity,
                             accum_out=ssum[:, 0:1])
        nc.scalar.activation(out=gscr[:], in_=xb[:, s3, :], func=AF.Square,
                             accum_out=ssum[:, 1:2])
        nc.vector.tensor_scalar(out=st[:, s3, :], in0=ssum[:], scalar1=1.0 / L, scalar2=None,
                                op0=ALU.mult)

        psg = psD.tile([2, 2 * CS], f32, tag="d")   # [group-half, (s, stat)]
        nc.tensor.matmul(psg[:], lhsT=g0[:], rhs=st[:].rearrange("p a b -> p (a b)"),
                         start=True, stop=True)
        pst = small.tile([2, 2 * CS], f32)
        nc.vector.tensor_copy(out=pst[:], in_=psg[:])
        pstv = pst[:].rearrange("p (s k) -> p s k", k=2)
        msq = small.tile([2, CS], f32)
        nc.vector.tensor_tensor(out=msq[:], in0=pstv[:, :, 0], in1=pstv[:, :, 0], op=ALU.mult)
        grp = small.tile([2, 2 * CS], f32)     # [:, :CS]=rstd_g, [:, CS:]=mean_g
        nc.vector.tensor_tensor(out=grp[:, 0:CS], in0=pstv[:, :, 1], in1=msq[:], op=ALU.subtract)
        nc.scalar.activation(out=grp[:, 0:CS], in_=grp[:, 0:CS], func=AF.Sqrt,
                             bias=eps2[:], scale=1.0)
        nc.vector.reciprocal(out=grp[:, 0:CS], in_=grp[:, 0:CS])
        nc.vector.tensor_copy(out=grp[:, CS:], in_=pstv[:, :, 0])
        # dummy Exp: pulls the Exp table-set load (~2.7us) off the first real
        # exp's critical path; Identity (used by the Q/K copies) is a filler
        # function present in every set.
        nc.scalar.activation(out=msq[:, 0:1], in_=eps2[:], func=AF.Exp)

        psbc = psD.tile([P, 2 * CS], f32, tag="d")  # broadcast groups -> channels
        nc.tensor.matmul(psbc[:], lhsT=sel[:], rhs=grp[:], start=True, stop=True)
        ab = small.tile([P, 2 * CS], f32)      # [:, :CS]=a_c, [:, CS:]=b_c
        nc.vector.tensor_tensor(out=ab[:, 0:CS], in0=GAM, in1=psbc[:, 0:CS], op=ALU.mult)
        nc.vector.tensor_tensor(out=ab[:, CS:], in0=psbc[:, CS:], in1=ab[:, 0:CS], op=ALU.mult)
        nc.vector.tensor_tensor(out=ab[:, CS:], in0=BET, in1=ab[:, CS:], op=ALU.subtract)
        # residual pass scalars with the output-projection bias folded in:
        # out = (a*x + b + bp) + Z/denom
        ab2 = small.tile([P, CS], f32)
        nc.vector.tensor_tensor(out=ab2[:], in0=ab[:, CS:], in1=BP, op=ALU.add)

        # ---- normalize: h^T = a*x^T + b ----
        # bf16 copy first (it gates all matmuls), split across DVE and ACT so
        # all four subtiles are ready ~2x sooner; the f32 in-place pass only
        # feeds the residual add much later, so it runs off the critical path.
        for s in range(CS):
            if s < 2:
                nc.vector.tensor_scalar(out=hb[:, s, :], in0=xb[:, s, :],
                                        scalar1=ab[:, s:s + 1], scalar2=ab[:, CS + s:CS + s + 1],
                                        op0=ALU.mult, op1=ALU.add)
            else:
                nc.scalar.activation(out=hb[:, s, :], in_=xb[:, s, :], func=AF.Identity,
                                     bias=ab[:, CS + s:CS + s + 1], scale=ab[:, s:s + 1])
        def residual_pass(lt):
            # h^T + bp for the lt-th finale, in place over the streamed f32 x
            for s in range(CS):
                nc.vector.tensor_scalar(out=xt[:, s, ts(lt, LQ)], in0=xt[:, s, ts(lt, LQ)],
                                        scalar1=ab[:, s:s + 1], scalar2=ab2[:, s:s + 1],
                                        op0=ALU.mult, op1=ALU.add)

        # ---- projections ----
        def project_t(w, bias, dst):
            # dst[:, co_s, l] = sum_ci w[ci, co]^T h^T + bias[co]
            for co_s in range(CS):
                for lt in range(NLT):
                    ps = psS.tile([P, LQ], f32, tag="s", name="ps_prj")
                    for ci in range(CS):
                        nc.tensor.matmul(ps[:], lhsT=w[:, ci, ts(co_s, P)],
                                         rhs=hb[:, ci, ts(lt, LQ)],
                                         start=(ci == 0), stop=(ci == CS - 1))
                    nc.scalar.activation(out=dst[:, co_s, ts(lt, LQ)], in_=ps[:],
                                         func=AF.Identity, bias=bias[:, co_s:co_s + 1], scale=1.0)

        project_t(wsb["wq"], BQ, qt)
        project_t(wsb["wk"], BK, kt)

        for lb in range(NLB):
            ps = psS.tile([P, C], f32, tag="s", name="ps_v")
            for ci in range(CS):
                nc.tensor.matmul(ps[:], lhsT=hb[:, ci, ts(lb, P)],
                                 rhs=wsb["wv"][:, ci, :],
                                 start=(ci == 0), stop=(ci == CS - 1))
            nc.vector.tensor_add(out=vt[:, lb, :], in0=ps[:], in1=bvb[:])

        # ---- attention + output projection, per lq tile ----
        for lt in range(NLT):
            po = [psA.tile([P, LQ], f32, tag="po", name=f"po{i}") for i in range(CS)]
            pd = psD.tile([1, LQ], f32, tag="d", name="pd")
            for kb in range(NLB):
                ps = psS.tile([P, LQ], f32, tag="s", name="ps_s")
                for ci in range(CS):
                    nc.tensor.matmul(ps[:], lhsT=kt[:, ci, ts(kb, P)],
                                     rhs=qt[:, ci, ts(lt, LQ)],
                                     start=(ci == 0), stop=(ci == CS - 1))
                pt = ptp.tile([P, LQ], bf16, tag="pt")
                nc.scalar.activation(out=pt[:], in_=ps[:], func=AF.Exp)
                for c_ in range(CS):
                    nc.tensor.matmul(po[c_][:], lhsT=vt[:, kb, ts(c_, P)], rhs=pt[:],
                                     start=(kb == 0), stop=(kb == NLB - 1))
                nc.tensor.matmul(pd[:], lhsT=ones_col[:], rhs=pt[:],
                                 start=(kb == 0), stop=(kb == NLB - 1))

            # Finale. Order matters: pdc frees the "d" bank and the ou copies
            # free the "po" banks that the next lq tile's denominator/PV
            # matmuls need — emit them first so DVE runs them first.
            # Broadcast raw denominators across partitions via PE, then take
            # the reciprocal on all 128 lanes (a [1,512] single-lane
            # reciprocal is ~2.7us and stalls the PE).
            pdc = small.tile([1, LQ], bf16, tag="pdc", bufs=2)
            with nc.allow_low_precision(reason="denom rounded to bf16 as matmul operand"):
                nc.vector.tensor_copy(out=pdc[:], in_=pd[:])
            ou = [oup.tile([P, LQ], bf16, tag="ou", name=f"ou{i}") for i in range(CS)]
            for c_ in range(CS):
                nc.scalar.copy(out=ou[c_][:], in_=po[c_][:])
            residual_pass(lt)
            # On the last tile nothing follows, so use the idle "s" slots and
            # let the finale matmuls/DVE run with full double-buffering.
            fin_ps, fin_tag = (psA, "po") if lt < NLT - 1 else (psS, "s")
            pb = fin_ps.tile([P, LQ], f32, tag=fin_tag, name="ps_b")
            nc.tensor.matmul(pb[:], lhsT=ones_row[:], rhs=pdc[:], start=True, stop=True)
            rb = finp.tile([P, LQ], f32, tag="rb")
            nc.vector.reciprocal(out=rb[:], in_=pb[:])

            for co_s in range(CS):
                pz = fin_ps.tile([P, LQ], f32, tag=fin_tag, name="ps_z")
                for ci in range(CS):
                    nc.tensor.matmul(pz[:], lhsT=wsb["wp"][:, ci, ts(co_s, P)],
                                     rhs=ou[ci][:],
                                     start=(ci == 0), stop=(ci == CS - 1))
                fin = finp.tile([P, LQ], f32, tag="fin")
                nc.vector.tensor_tensor(out=fin[:], in0=pz[:], in1=rb[:], op=ALU.mult)
                nc.vector.tensor_tensor(out=fin[:], in0=fin[:],
                                        in1=xt[:, co_s, ts(lt, LQ)], op=ALU.add)
                nc.sync.dma_start(out=out_dv[:, co_s, ts(lt, LQ)], in_=fin[:])

    nc.compile()
    return nc


def get_nc():
    if "nc" not in _CACHE:
        _CACHE["nc"] = _build_nc()
    return _CACHE["nc"]


def _g0_const():
    g = np.zeros((P, 2), np.float32)
    g[0:CPG, 0] = 1.0 / CPG
    g[CPG:P, 1] = 1.0 / CPG
    return g


def _sel_const():
    s = np.zeros((2, P), np.float32)
    s[0, 0:CPG] = 1.0
    s[1, CPG:P] = 1.0
    return s


def prep_inputs(x, gamma, beta, wq, bq, wk, bk, wv, bv, wp, bp):
    """Host-side layout prep (transposes / reshapes / bf16 weight casts, plus
    folding the 1/sqrt(C) attention scale into wk/bk). Per-core input maps."""
    import ml_dtypes

    f = np.float32
    bf = ml_dtypes.bfloat16
    x = np.asarray(x, f)
    scale = f(C) ** f(-0.5)

    def wprep(w):
        w = np.asarray(w, f)
        return np.ascontiguousarray(w.reshape(CS, P, C).transpose(1, 0, 2)).astype(bf)

    def vprep(v):
        v = np.asarray(v, f)
        return np.ascontiguousarray(v.reshape(CS, P).T)

    shared = {
        "wq": wprep(wq), "wk": wprep(np.asarray(wk, f) * scale),
        "wv": wprep(wv), "wp": wprep(wp),
        "vp": np.ascontiguousarray(np.concatenate(
            [vprep(gamma), vprep(beta), vprep(bq),
             vprep(np.asarray(bk, f) * scale), vprep(bp)], axis=1)),
        "bv_bcast": np.ascontiguousarray(np.broadcast_to(np.asarray(bv, f), (P, C))),
        "g0": _g0_const(), "sel": _sel_const(),
    }
    in_maps = []
    for b in range(N_CORES):
        m = dict(shared)
        xtb = np.ascontiguousarray(x[b].T)                       # [C, L]
        m["xb"] = xtb.astype(bf)
        # [NLT, P, CS, LQ]: per-lq-tile chunks of x^T in [p, s, j] layout
        m["xr"] = np.ascontiguousarray(
            xtb.reshape(CS, P, NLT, LQ).transpose(2, 1, 0, 3))
        in_maps.append(m)
    return in_maps


def run(inputs, trace=False, **kw):
    from concourse.bass_utils import run_bass_kernel_spmd

    nc = get_nc()
    in_maps = prep_inputs(**inputs)
    return run_bass_kernel_spmd(nc, in_maps, core_ids=list(range(N_CORES)),
                                trace=trace, **kw)


def kernel(**inputs) -> np.ndarray:
    res = run(inputs)
    out = np.empty((B, L, C), np.float32)
    for b in range(N_CORES):
        out[b] = res.results[b]["out_t"].T
    return out


# revision 46
# speedup vs baseline: 1.2021x; 1.2021x over previous
"""Trainium2 Bass kernel for nn_AttentionBlock (B=8, L=2048, C=512, GroupNorm(8) +
single-head attention + residual), data-parallel over batch across 8 NeuronCores.

Self-contained: hardcodes shapes/sharding. kernel(**inputs) -> np.ndarray [B,L,C].

Dataflow (per core, one batch element, everything channel-major / "transposed"):
  x^T [C,L] (f32) --bn_stats/group-reduce--> h^T = a_c * x^T + b_c  (f32 + bf16 copy)
  Q^T = wq^T h^T + bq ;  K^T = (wk*scale)^T h^T + bk*scale  (scale folded on host)
  V   = h^T-chunks^T @ wv + bv           (natural [L,C] layout)
  per 512-wide lq tile:
     for each 128-key block: S^T = K^T-chunk^T @ Q^T (PSUM f32); P = exp(S^T) (bf16)
     O^T  += V-chunk^T @ P  (PSUM f32 accum over key blocks), denom += 1^T @ P
     out^T = h^T + (wp^T O^T) * (1/denom) + bp      (f32 combine)
Matmul operands are bf16 (1 cyc/row on PE); accumulation always fp32 in PSUM.
Host side transposes x per batch, casts weights to bf16, transposes output back.
"""

import numpy as np

B, L, C = 8, 2048, 512
GROUPS = 8
EPS = 1e-3
P = 128
CS = C // P            # 4 channel subtiles of 128
LQ = 512               # lq tile width (matmul free dim)
NLT = L // LQ          # 4 lq tiles
NLB = L // P           # 16 key/l blocks
CPG = C // GROUPS      # 64 channels per group
N_CORES = 8

_CACHE = {}


def _build_nc():
    from contextlib import ExitStack

    import concourse.bass as bass
    import concourse.mybir as mybir
    import concourse.tile as tile
    from concourse import bacc
    from concourse.bass import ts

    f32 = mybir.dt.float32
    bf16 = mybir.dt.bfloat16
    AF = mybir.ActivationFunctionType
    ALU = mybir.AluOpType

    nc = bacc.Bacc(trn_type="TRN2")

    xr_d = nc.dram_tensor("xr", [NLT, P, CS, LQ], f32, kind="ExternalInput")
    xb_d = nc.dram_tensor("xb", [C, L], bf16, kind="ExternalInput")
    w_d = {
        n: nc.dram_tensor(n, [P, CS, C], bf16, kind="ExternalInput")
        for n in ("wq", "wk", "wv", "wp")
    }
    # packed per-channel vectors: [gamma, beta, bq, bk, bp] x CS columns
    vp_d = nc.dram_tensor("vp", [P, 5 * CS], f32, kind="ExternalInput")
    bvb_d = nc.dram_tensor("bv_bcast", [P, C], f32, kind="ExternalInput")
    g0_d = nc.dram_tensor("g0", [P, 2], f32, kind="ExternalInput")
    sel_d = nc.dram_tensor("sel", [2, P], f32, kind="ExternalInput")
    out_d = nc.dram_tensor("out_t", [C, L], f32, kind="ExternalOutput")

    xb_dv = xb_d[:].rearrange("(s p) l -> p s l", p=P)
    out_dv = out_d[:].rearrange("(s p) l -> p s l", p=P)

    with tile.TileContext(nc) as tc, ExitStack() as ctx:
        consts = ctx.enter_context(tc.tile_pool(name="consts", bufs=1))
        data = ctx.enter_context(tc.tile_pool(name="data", bufs=1))
        small = ctx.enter_context(tc.tile_pool(name="small", bufs=1))
        ptp = ctx.enter_context(tc.tile_pool(name="ptp", bufs=3))
        oup = ctx.enter_context(tc.tile_pool(name="oup", bufs=4))
        finp = ctx.enter_context(tc.tile_pool(name="finp", bufs=2))
        psA = ctx.enter_context(tc.tile_pool(name="psA", bufs=4, space="PSUM"))
        psS = ctx.enter_context(tc.tile_pool(name="psS", bufs=3, space="PSUM"))
        psD = ctx.enter_context(tc.tile_pool(name="psD", bufs=1, space="PSUM"))

        # ---- SBUF residents ----
        xt = data.tile([P, CS, L], f32)       # x^T, then h^T (f32, residual)
        xb = data.tile([P, CS, L], bf16)      # x^T bf16, stats fast path
        hb = data.tile([P, CS, L], bf16)      # h^T bf16 (matmul operand)
        qt = data.tile([P, CS, L], bf16)      # Q^T
        kt = data.tile([P, CS, L], bf16)      # K^T (pre-scaled)
        vt = data.tile([P, NLB, C], bf16)     # V natural, [l%P, l//P, c]
        wsb = {n: consts.tile([P, CS, C], bf16, name=f"w_{n}") for n in w_d}
        vp = consts.tile([P, 5 * CS], f32)
        GAM, BET, BQ, BK, BP = (vp[:, i * CS:(i + 1) * CS] for i in range(5))
        bvb = consts.tile([P, C], f32)
        g0 = consts.tile([P, 2], f32)
        sel = consts.tile([2, P], f32)
        ones_col = consts.tile([P, 1], bf16)
        ones_row = consts.tile([1, P], bf16)
        eps2 = consts.tile([2, 1], f32)

        # ---- loads + constants ----
        # DMA order IS the critical path: tiny constants first (they gate the
        # stats reduce), then bf16 x (gates bn_stats), then weights (gate the
        # first projection matmuls), then f32 x (residual only — needed late).
        nc.gpsimd.dma_start(out=g0[:], in_=g0_d[:])
        nc.gpsimd.dma_start(out=sel[:], in_=sel_d[:])
        nc.gpsimd.dma_start(out=vp[:], in_=vp_d[:])
        for s in range(CS):
            nc.sync.dma_start(out=xb[:, s, :], in_=xb_dv[:, s, :])
        for n in ("wq", "wk", "wv", "wp"):
            nc.sync.dma_start(out=wsb[n][:], in_=w_d[n][:])
        nc.sync.dma_start(out=bvb[:], in_=bvb_d[:])
        # f32 x only feeds the residual add in the lt-th finale — stream it
        # per lq tile so it never competes with the latency-critical loads.
        for lt in range(NLT):
            nc.sync.dma_start(out=xt[:, :, ts(lt, LQ)], in_=xr_d[lt])
        nc.vector.memset(ones_col[:], 1.0)
        nc.vector.memset(ones_row[:], 1.0)
        nc.vector.memset(eps2[:], EPS)

        # ---- GroupNorm stats ----
        # per-channel (partition) mean/var over L via bn_stats, then group
        # aggregation across partitions with a tiny fp32 matmul.
        st = small.tile([P, CS, 2], f32)      # (mean_c, E[x^2]_c) per subtile
        # subtiles 0..2 on DVE (bn_stats); subtile 3 on the otherwise-idle ACT
        # via activation accum_out (sum and sum-of-squares along L).
        for s in range(CS - 1):
            st6 = small.tile([P, 4, 6], f32, tag="st6", bufs=2)
            for j in range(4):
                nc.vector.bn_stats(out=st6[:, j, :], in_=xb[:, s, ts(j, 512)])
            mv = small.tile([P, 2], f32, tag="mv", bufs=2)
            nc.vector.bn_aggr(out=mv[:], in_=st6[:])
            nc.vector.tensor_copy(out=st[:, s, 0:1], in_=mv[:, 0:1])
            nc.vector.tensor_tensor(out=st[:, s, 1:2], in0=mv[:, 0:1], in1=mv[:, 0:1], op=ALU.mult)
            nc.vector.tensor_tensor(out=st[:, s, 1:2], in0=st[:, s, 1:2], in1=mv[:, 1:2], op=ALU.add)
        s3 = CS - 1
        ssum = small.tile([P, 2], f32)        # (sum, sumsq) of subtile 3
        gscr = small.tile([P, L], bf16)
        nc.scalar.activation(out=gscr[:], in_=xb[:, s3, :], func=AF.Identity,
                             accum_out=ssum[:, 0:1])
        nc.scalar.activation(out=gscr[:], in_=xb[:, s3, :], func=AF.Square,
                             accum_out=ssum[:, 1:2])
        nc.vector.tensor_scalar(out=st[:, s3, :], in0=ssum[:], scalar1=1.0 / L, scalar2=None,
                                op0=ALU.mult)

        psg = psD.tile([2, 2 * CS], f32, tag="d")   # [group-half, (s, stat)]
        nc.tensor.matmul(psg[:], lhsT=g0[:], rhs=st[:].rearrange("p a b -> p (a b)"),
                         start=True, stop=True)
        pst = small.tile([2, 2 * CS], f32)
        nc.vector.tensor_copy(out=pst[:], in_=psg[:])
        pstv = pst[:].rearrange("p (s k) -> p s k", k=2)
        msq = small.tile([2, CS], f32)
        nc.vector.tensor_tensor(out=msq[:], in0=pstv[:, :, 0], in1=pstv[:, :, 0], op=ALU.mult)
        grp = small.tile([2, 2 * CS], f32)     # [:, :CS]=rstd_g, [:, CS:]=mean_g
        nc.vector.tensor_tensor(out=grp[:, 0:CS], in0=pstv[:, :, 1], in1=msq[:], op=ALU.subtract)
        nc.scalar.activation(out=grp[:, 0:CS], in_=grp[:, 0:CS], func=AF.Sqrt,
                             bias=eps2[:], scale=1.0)
        nc.vector.reciprocal(out=grp[:, 0:CS], in_=grp[:, 0:CS])
        nc.vector.tensor_copy(out=grp[:, CS:], in_=pstv[:, :, 0])
        # dummy Exp: pulls the Exp table-set load (~2.7us) off the first real
        # exp's critical path; Identity (used by the Q/K copies) is a filler
        # function present in every set.
        nc.scalar.activation(out=msq[:, 0:1], in_=eps2[:], func=AF.Exp)

        psbc = psD.tile([P, 2 * CS], f32, tag="d")  # broadcast groups -> channels
        nc.tensor.matmul(psbc[:], lhsT=sel[:], rhs=grp[:], start=True, stop=True)
        ab = small.tile([P, 2 * CS], f32)      # [:, :CS]=a_c, [:, CS:]=b_c
        nc.vector.tensor_tensor(out=ab[:, 0:CS], in0=GAM, in1=psbc[:, 0:CS], op=ALU.mult)
        nc.vector.tensor_tensor(out=ab[:, CS:], in0=psbc[:, CS:], in1=ab[:, 0:CS], op=ALU.mult)
        nc.vector.tensor_tensor(out=ab[:, CS:], in0=BET, in1=ab[:, CS:], op=ALU.subtract)
        # residual pass scalars with the output-projection bias folded in:
        # out = (a*x + b + bp) + Z/denom
        ab2 = small.tile([P, CS], f32)
        nc.vector.tensor_tensor(out=ab2[:], in0=ab[:, CS:], in1=BP, op=ALU.add)

        # ---- normalize: h^T = a*x^T + b ----
        # bf16 copy first (it gates all matmuls), split across DVE and ACT so
        # all four subtiles are ready ~2x sooner; the f32 in-place pass only
        # feeds the residual add much later, so it runs off the critical path.
        for s in range(CS):
            if s < 2:
                nc.vector.tensor_scalar(out=hb[:, s, :], in0=xb[:, s, :],
                                        scalar1=ab[:, s:s + 1], scalar2=ab[:, CS + s:CS + s + 1],
                                        op0=ALU.mult, op1=ALU.add)
            else:
                nc.scalar.activation(out=hb[:, s, :], in_=xb[:, s, :], func=AF.Identity,
                                     bias=ab[:, CS + s:CS + s + 1], scale=ab[:, s:s + 1])
        def residual_pass(lt):
            # h^T + bp for the lt-th finale, in place over the streamed f32 x
            for s in range(CS):
                nc.vector.tensor_scalar(out=xt[:, s, ts(lt, LQ)], in0=xt[:, s, ts(lt, LQ)],
                                        scalar1=ab[:, s:s + 1], scalar2=ab2[:, s:s + 1],
                                        op0=ALU.mult, op1=ALU.add)

        # ---- projections ----
        def project_t(w, bias, dst):
            # dst[:, co_s, l] = sum_ci w[ci, co]^T h^T + bias[co]
            for co_s in range(CS):
                for lt in range(NLT):
                    ps = psS.tile([P, LQ], f32, tag="s", name="ps_prj")
                    for ci in range(CS):
                        nc.tensor.matmul(ps[:], lhsT=w[:, ci, ts(co_s, P)],
                                         rhs=hb[:, ci, ts(lt, LQ)],
                                         start=(ci == 0), stop=(ci == CS - 1))
                    nc.scalar.activation(out=dst[:, co_s, ts(lt, LQ)], in_=ps[:],
                                         func=AF.Identity, bias=bias[:, co_s:co_s + 1], scale=1.0)

        project_t(wsb["wq"], BQ, qt)
        project_t(wsb["wk"], BK, kt)

        for lb in range(NLB):
            ps = psS.tile([P, C], f32, tag="s", name="ps_v")
            for ci in range(CS):
                nc.tensor.matmul(ps[:], lhsT=hb[:, ci, ts(lb, P)],
                                 rhs=wsb["wv"][:, ci, :],
                                 start=(ci == 0), stop=(ci == CS - 1))
            nc.vector.tensor_add(out=vt[:, lb, :], in0=ps[:], in1=bvb[:])

        # ---- attention + output projection, per lq tile ----
        for lt in range(NLT):
            po = [psA.tile([P, LQ], f32, tag="po", name=f"po{i}") for i in range(CS)]
            pd = psD.tile([1, LQ], f32, tag="d", name="pd")
            for kb in range(NLB):
                ps = psS.tile([P, LQ], f32, tag="s", name="ps_s")
                for ci in range(CS):
                    nc.tensor.matmul(ps[:], lhsT=kt[:, ci, ts(kb, P)],
                                     rhs=qt[:, ci, ts(lt, LQ)],
                                     start=(ci == 0), stop=(ci == CS - 1))
                pt = ptp.tile([P, LQ], bf16, tag="pt")
                nc.scalar.activation(out=pt[:], in_=ps[:], func=AF.Exp)
                for c_ in range(CS):
                    nc.tensor.matmul(po[c_][:], lhsT=vt[:, kb, ts(c_, P)], rhs=pt[:],
                                     start=(kb == 0), stop=(kb == NLB - 1))
                nc.tensor.matmul(pd[:], lhsT=ones_col[:], rhs=pt[:],
                                 start=(kb == 0), stop=(kb == NLB - 1))

            # Finale. Order matters: pdc frees the "d" bank and the ou copies
            # free the "po" banks that the next lq tile's denominator/PV
            # matmuls need — emit them first so DVE runs them first.
            # Broadcast raw denominators across partitions via PE, then take
            # the reciprocal on all 128 lanes (a [1,512] single-lane
            # reciprocal is ~2.7us and stalls the PE).
            pdc = small.tile([1, LQ], bf16, tag="pdc", bufs=2)
            with nc.allow_low_precision(reason="denom rounded to bf16 as matmul operand"):
                nc.vector.tensor_copy(out=pdc[:], in_=pd[:])
            ou = [oup.tile([P, LQ], bf16, tag="ou", name=f"ou{i}") for i in range(CS)]
            for c_ in range(CS):
                nc.vector.tensor_copy(out=ou[c_][:], in_=po[c_][:])
            residual_pass(lt)
            # On the last tile nothing follows, so use the idle "s" slots and
            # let the finale matmuls/DVE run with full double-buffering.
            fin_ps, fin_tag = (psA, "po") if lt < NLT - 1 else (psS, "s")
            pb = fin_ps.tile([P, LQ], f32, tag=fin_tag, name="ps_b")
            nc.tensor.matmul(pb[:], lhsT=ones_row[:], rhs=pdc[:], start=True, stop=True)
            rb = finp.tile([P, LQ], f32, tag="rb")
            nc.vector.reciprocal(out=rb[:], in_=pb[:])

            for co_s in range(CS):
                pz = fin_ps.tile([P, LQ], f32, tag=fin_tag, name="ps_z")
                for ci in range(CS):
                    nc.tensor.matmul(pz[:], lhsT=wsb["wp"][:, ci, ts(co_s, P)],
                                     rhs=ou[ci][:],
                                     start=(ci == 0), stop=(ci == CS - 1))
                fin = finp.tile([P, LQ], f32, tag="fin")
                nc.vector.tensor_tensor(out=fin[:], in0=pz[:], in1=rb[:], op=ALU.mult)
                nc.vector.tensor_tensor(out=fin[:], in0=fin[:],
                                        in1=xt[:, co_s, ts(lt, LQ)], op=ALU.add)
                nc.sync.dma_start(out=out_dv[:, co_s, ts(lt, LQ)], in_=fin[:])

    nc.compile()
    return nc


def get_nc():
    if "nc" not in _CACHE:
        _CACHE["nc"] = _build_nc()
    return _CACHE["nc"]


def _g0_const():
    g = np.zeros((P, 2), np.float32)
    g[0:CPG, 0] = 1.0 / CPG
    g[CPG:P, 1] = 1.0 / CPG
    return g


def _sel_const():
    s = np.zeros((2, P), np.float32)
    s[0, 0:CPG] = 1.0
    s[1, CPG:P] = 1.0
    return s


def prep_inputs(x, gamma, beta, wq, bq, wk, bk, wv, bv, wp, bp):
    """Host-side layout prep (transposes / reshapes / bf16 weight casts, plus
    folding the 1/sqrt(C) attention scale into wk/bk). Per-core input maps."""
    import ml_dtypes

    f = np.float32
    bf = ml_dtypes.bfloat16
    x = np.asarray(x, f)
    scale = f(C) ** f(-0.5)

    def wprep(w):
        w = np.asarray(w, f)
        return np.ascontiguousarray(w.reshape(CS, P, C).transpose(1, 0, 2)).astype(bf)

    def vprep(v):
        v = np.asarray(v, f)
        return np.ascontiguousarray(v.reshape(CS, P).T)

    shared = {
        "wq": wprep(wq), "wk": wprep(np.asarray(wk, f) * scale),
        "wv": wprep(wv), "wp": wprep(wp),
        "vp": np.ascontiguousarray(np.concatenate(
            [vprep(gamma), vprep(beta), vprep(bq),
             vprep(np.asarray(bk, f) * scale), vprep(bp)], axis=1)),
        "bv_bcast": np.ascontiguousarray(np.broadcast_to(np.asarray(bv, f), (P, C))),
        "g0": _g0_const(), "sel": _sel_const(),
    }
    in_maps = []
    for b in range(N_CORES):
        m = dict(shared)
        xtb = np.ascontiguousarray(x[b].T)                       # [C, L]
        m["xb"] = xtb.astype(bf)
        # [NLT, P, CS, LQ]: per-lq-tile chunks of x^T in [p, s, j] layout
        m["xr"] = np.ascontiguousarray(
            xtb.reshape(CS, P, NLT, LQ).transpose(2, 1, 0, 3))
        in_maps.append(m)
    return in_maps


def run(inputs, trace=False, **kw):
    from concourse.bass_utils import run_bass_kernel_spmd

    nc = get_nc()
    in_maps = prep_inputs(**inputs)
    return run_bass_kernel_spmd(nc, in_maps, core_ids=list(range(N_CORES)),
                                trace=trace, **kw)


def kernel(**inputs) -> np.ndarray:
    res = run(inputs)
    out = np.empty((B, L, C), np.float32)
    for b in range(N_CORES):
        out[b] = res.results[b]["out_t"].T
    return out


# revision 50
# speedup vs baseline: 1.6684x; 1.3879x over previous
"""Trainium2 Bass kernel for nn_AttentionBlock (B=8, L=2048, C=512, GroupNorm(8) +
single-head attention + residual), data-parallel over batch across 8 NeuronCores.

Self-contained: hardcodes shapes/sharding. kernel(**inputs) -> np.ndarray [B,L,C].

Dataflow (per core, one batch element, everything channel-major / "transposed"):
  x^T [C,L] (f32) --bn_stats/group-reduce--> h^T = a_c * x^T + b_c  (f32 + bf16 copy)
  Q^T = wq^T h^T + bq ;  K^T = (wk*scale)^T h^T + bk*scale  (scale folded on host)
  V   = h^T-chunks^T @ wv + bv           (natural [L,C] layout)
  per 512-wide lq tile:
     for each 128-key block: S^T = K^T-chunk^T @ Q^T (PSUM f32); P = exp(S^T) (bf16)
     O^T  += V-chunk^T @ P  (PSUM f32 accum over key blocks), denom += 1^T @ P
     out^T = h^T + (wp^T O^T) * (1/denom) + bp      (f32 combine)
Matmul operands are bf16 (1 cyc/row on PE); accumulation always fp32 in PSUM.
Host side transposes x per batch, casts weights to bf16, transposes output back.
"""

import numpy as np

B, L, C = 8, 2048, 512
GROUPS = 8
EPS = 1e-3
P = 128
CS = C // P            # 4 channel subtiles of 128
LQ = 512               # lq tile width (matmul free dim)
NLT = L // LQ          # 4 lq tiles
NLB = L // P           # 16 key/l blocks
CPG = C // GROUPS      # 64 channels per group
N_CORES = 8

_CACHE = {}


def _build_nc():
    from contextlib import ExitStack

    import concourse.bass as bass
    import concourse.mybir as mybir
    import concourse.tile as tile
    from concourse import bacc
    from concourse.bass import ts

    f32 = mybir.dt.float32
    bf16 = mybir.dt.bfloat16
    fp8 = mybir.dt.float8e4
    DR = mybir.MatmulPerfMode.DoubleRow
    AF = mybir.ActivationFunctionType
    ALU = mybir.AluOpType

    nc = bacc.Bacc(trn_type="TRN2")

    xr_d = nc.dram_tensor("xr", [NLT, P, CS, LQ], f32, kind="ExternalInput")
    xb_d = nc.dram_tensor("xb", [C, L], bf16, kind="ExternalInput")
    w_d = {
        n: nc.dram_tensor(n, [P, CS, C], bf16, kind="ExternalInput")
        for n in ("wq", "wk", "wv", "wp")
    }
    # packed per-channel vectors: [gamma, beta, bq, bk, bp] x CS columns
    vp_d = nc.dram_tensor("vp", [P, 5 * CS], f32, kind="ExternalInput")
    bvb_d = nc.dram_tensor("bv_bcast", [P, C], f32, kind="ExternalInput")
    g0_d = nc.dram_tensor("g0", [P, 2], f32, kind="ExternalInput")
    sel_d = nc.dram_tensor("sel", [2, P], f32, kind="ExternalInput")
    out_d = nc.dram_tensor("out_t", [C, L], f32, kind="ExternalOutput")

    xb_dv = xb_d[:].rearrange("(s p) l -> p s l", p=P)
    out_dv = out_d[:].rearrange("(s p) l -> p s l", p=P)

    with tile.TileContext(nc) as tc, ExitStack() as ctx:
        consts = ctx.enter_context(tc.tile_pool(name="consts", bufs=1))
        data = ctx.enter_context(tc.tile_pool(name="data", bufs=1))
        small = ctx.enter_context(tc.tile_pool(name="small", bufs=1))
        ptp = ctx.enter_context(tc.tile_pool(name="ptp", bufs=3))
        oup = ctx.enter_context(tc.tile_pool(name="oup", bufs=4))
        finp = ctx.enter_context(tc.tile_pool(name="finp", bufs=2))
        psA = ctx.enter_context(tc.tile_pool(name="psA", bufs=4, space="PSUM"))
        psS = ctx.enter_context(tc.tile_pool(name="psS", bufs=3, space="PSUM"))
        psD = ctx.enter_context(tc.tile_pool(name="psD", bufs=1, space="PSUM"))

        # ---- SBUF residents ----
        xt = data.tile([P, CS, L], f32)       # x^T, then h^T (f32, residual)
        xb = data.tile([P, CS, L], bf16)      # x^T bf16, stats fast path
        hb = data.tile([P, CS, L], bf16)      # h^T bf16 (matmul operand)
        qt = data.tile([P, CS, L], fp8)       # Q^T (fp8: attention matmuls run
        kt = data.tile([P, CS, L], fp8)       # DoubleRow, 2x PE throughput)
        vt = data.tile([P, NLB, C], fp8)      # V natural, [l%P, l//P, c]
        wsb = {n: consts.tile([P, CS, C], bf16, name=f"w_{n}") for n in w_d}
        vp = consts.tile([P, 5 * CS], f32)
        GAM, BET, BQ, BK, BP = (vp[:, i * CS:(i + 1) * CS] for i in range(5))
        bvb = consts.tile([P, C], f32)
        g0 = consts.tile([P, 2], f32)
        sel = consts.tile([2, P], f32)
        ones_col = consts.tile([P, 2, 16], fp8)   # [:, :, 0:1] = DoubleRow ones
        ones_row = consts.tile([1, P], bf16)
        eps2 = consts.tile([2, 1], f32)

        # ---- loads + constants ----
        # DMA order IS the critical path: tiny constants first (they gate the
        # stats reduce), then bf16 x (gates bn_stats), then weights (gate the
        # first projection matmuls), then f32 x (residual only — needed late).
        nc.gpsimd.dma_start(out=g0[:], in_=g0_d[:])
        nc.gpsimd.dma_start(out=sel[:], in_=sel_d[:])
        nc.gpsimd.dma_start(out=vp[:], in_=vp_d[:])
        for s in range(CS):
            nc.sync.dma_start(out=xb[:, s, :], in_=xb_dv[:, s, :])
        for n in ("wq", "wk", "wv", "wp"):
            nc.sync.dma_start(out=wsb[n][:], in_=w_d[n][:])
        nc.sync.dma_start(out=bvb[:], in_=bvb_d[:])
        # f32 x only feeds the residual add in the lt-th finale — stream it
        # per lq tile so it never competes with the latency-critical loads.
        for lt in range(NLT):
            nc.sync.dma_start(out=xt[:, :, ts(lt, LQ)], in_=xr_d[lt])
        nc.vector.memset(ones_col[:], 1.0)
        nc.vector.memset(ones_row[:], 1.0)
        nc.vector.memset(eps2[:], EPS)

        # ---- GroupNorm stats ----
        # per-channel (partition) mean/var over L via bn_stats, then group
        # aggregation across partitions with a tiny fp32 matmul.
        st = small.tile([P, CS, 2], f32)      # (mean_c, E[x^2]_c) per subtile
        # subtiles 0..2 on DVE (bn_stats); subtile 3 on the otherwise-idle ACT
        # via activation accum_out (sum and sum-of-squares along L).
        for s in range(CS - 1):
            st6 = small.tile([P, 4, 6], f32, tag="st6", bufs=2)
            for j in range(4):
                nc.vector.bn_stats(out=st6[:, j, :], in_=xb[:, s, ts(j, 512)])
            mv = small.tile([P, 2], f32, tag="mv", bufs=2)
            nc.vector.bn_aggr(out=mv[:], in_=st6[:])
            nc.vector.tensor_copy(out=st[:, s, 0:1], in_=mv[:, 0:1])
            nc.vector.tensor_tensor(out=st[:, s, 1:2], in0=mv[:, 0:1], in1=mv[:, 0:1], op=ALU.mult)
            nc.vector.tensor_tensor(out=st[:, s, 1:2], in0=st[:, s, 1:2], in1=mv[:, 1:2], op=ALU.add)
        s3 = CS - 1
        ssum = small.tile([P, 2], f32)        # (sum, sumsq) of subtile 3
        gscr = small.tile([P, L], bf16)
        nc.scalar.activation(out=gscr[:], in_=xb[:, s3, :], func=AF.Identity,
                             accum_out=ssum[:, 0:1])
        nc.scalar.activation(out=gscr[:], in_=xb[:, s3, :], func=AF.Square,
                             accum_out=ssum[:, 1:2])
        nc.vector.tensor_scalar(out=st[:, s3, :], in0=ssum[:], scalar1=1.0 / L, scalar2=None,
                                op0=ALU.mult)

        psg = psD.tile([2, 2 * CS], f32, tag="d")   # [group-half, (s, stat)]
        nc.tensor.matmul(psg[:], lhsT=g0[:], rhs=st[:].rearrange("p a b -> p (a b)"),
                         start=True, stop=True)
        pst = small.tile([2, 2 * CS], f32)
        nc.vector.tensor_copy(out=pst[:], in_=psg[:])
        pstv = pst[:].rearrange("p (s k) -> p s k", k=2)
        msq = small.tile([2, CS], f32)
        nc.vector.tensor_tensor(out=msq[:], in0=pstv[:, :, 0], in1=pstv[:, :, 0], op=ALU.mult)
        grp = small.tile([2, 2 * CS], f32)     # [:, :CS]=rstd_g, [:, CS:]=mean_g
        nc.vector.tensor_tensor(out=grp[:, 0:CS], in0=pstv[:, :, 1], in1=msq[:], op=ALU.subtract)
        nc.scalar.activation(out=grp[:, 0:CS], in_=grp[:, 0:CS], func=AF.Sqrt,
                             bias=eps2[:], scale=1.0)
        nc.vector.reciprocal(out=grp[:, 0:CS], in_=grp[:, 0:CS])
        nc.vector.tensor_copy(out=grp[:, CS:], in_=pstv[:, :, 0])
        # dummy Exp: pulls the Exp table-set load (~2.7us) off the first real
        # exp's critical path; Identity (used by the Q/K copies) is a filler
        # function present in every set.
        nc.scalar.activation(out=msq[:, 0:1], in_=eps2[:], func=AF.Exp)

        psbc = psD.tile([P, 2 * CS], f32, tag="d")  # broadcast groups -> channels
        nc.tensor.matmul(psbc[:], lhsT=sel[:], rhs=grp[:], start=True, stop=True)
        ab = small.tile([P, 2 * CS], f32)      # [:, :CS]=a_c, [:, CS:]=b_c
        nc.vector.tensor_tensor(out=ab[:, 0:CS], in0=GAM, in1=psbc[:, 0:CS], op=ALU.mult)
        nc.vector.tensor_tensor(out=ab[:, CS:], in0=psbc[:, CS:], in1=ab[:, 0:CS], op=ALU.mult)
        nc.vector.tensor_tensor(out=ab[:, CS:], in0=BET, in1=ab[:, CS:], op=ALU.subtract)
        # residual pass scalars with the output-projection bias folded in:
        # out = (a*x + b + bp) + Z/denom
        ab2 = small.tile([P, CS], f32)
        nc.vector.tensor_tensor(out=ab2[:], in0=ab[:, CS:], in1=BP, op=ALU.add)

        # ---- normalize: h^T = a*x^T + b ----
        # bf16 copy first (it gates all matmuls), split across DVE and ACT so
        # all four subtiles are ready ~2x sooner; the f32 in-place pass only
        # feeds the residual add much later, so it runs off the critical path.
        for s in range(CS):
            if s < 2:
                nc.vector.tensor_scalar(out=hb[:, s, :], in0=xb[:, s, :],
                                        scalar1=ab[:, s:s + 1], scalar2=ab[:, CS + s:CS + s + 1],
                                        op0=ALU.mult, op1=ALU.add)
            else:
                nc.scalar.activation(out=hb[:, s, :], in_=xb[:, s, :], func=AF.Identity,
                                     bias=ab[:, CS + s:CS + s + 1], scale=ab[:, s:s + 1])
        def residual_pass(lt):
            # h^T + bp for the lt-th finale, in place over the streamed f32 x
            for s in range(CS):
                nc.vector.tensor_scalar(out=xt[:, s, ts(lt, LQ)], in0=xt[:, s, ts(lt, LQ)],
                                        scalar1=ab[:, s:s + 1], scalar2=ab2[:, s:s + 1],
                                        op0=ALU.mult, op1=ALU.add)

        # ---- projections ----
        def project_t(w, bias, dst):
            # dst[:, co_s, l] = sum_ci w[ci, co]^T h^T + bias[co]
            for co_s in range(CS):
                for lt in range(NLT):
                    ps = psS.tile([P, LQ], f32, tag="s", name="ps_prj")
                    for ci in range(CS):
                        nc.tensor.matmul(ps[:], lhsT=w[:, ci, ts(co_s, P)],
                                         rhs=hb[:, ci, ts(lt, LQ)],
                                         start=(ci == 0), stop=(ci == CS - 1))
                    nc.scalar.activation(out=dst[:, co_s, ts(lt, LQ)], in_=ps[:],
                                         func=AF.Identity, bias=bias[:, co_s:co_s + 1], scale=1.0)

        project_t(wsb["wq"], BQ, qt)
        project_t(wsb["wk"], BK, kt)

        for lb in range(NLB):
            ps = psS.tile([P, C], f32, tag="s", name="ps_v")
            for ci in range(CS):
                nc.tensor.matmul(ps[:], lhsT=hb[:, ci, ts(lb, P)],
                                 rhs=wsb["wv"][:, ci, :],
                                 start=(ci == 0), stop=(ci == CS - 1))
            nc.vector.tensor_add(out=vt[:, lb, :], in0=ps[:], in1=bvb[:])

        # ---- attention + output projection, per lq tile ----
        for lt in range(NLT):
            po = [psA.tile([P, LQ], f32, tag="po", name=f"po{i}") for i in range(CS)]
            pd = psD.tile([1, LQ], f32, tag="d", name="pd")
            for kp in range(NLB // 2):
                # S^T for a pair of key blocks: 2 DoubleRow matmuls each
                # (contraction 256 = two channel subtiles per matmul)
                pt2 = ptp.tile([P, 2, LQ], fp8, tag="pt")
                for i in range(2):
                    kb = 2 * kp + i
                    ps = psS.tile([P, LQ], f32, tag="s", name="ps_s")
                    for cp in range(2):
                        nc.tensor.matmul(ps[:], lhsT=kt[:, 2 * cp:2 * cp + 2, ts(kb, P)],
                                         rhs=qt[:, 2 * cp:2 * cp + 2, ts(lt, LQ)],
                                         start=(cp == 0), stop=(cp == 1), perf_mode=DR)
                    nc.scalar.activation(out=pt2[:, i, :], in_=ps[:], func=AF.Exp)
                # PV + denominator over the key-block pair, DoubleRow again
                for c_ in range(CS):
                    nc.tensor.matmul(po[c_][:], lhsT=vt[:, 2 * kp:2 * kp + 2, ts(c_, P)],
                                     rhs=pt2[:], start=(kp == 0), stop=(kp == NLB // 2 - 1),
                                     perf_mode=DR)
                nc.tensor.matmul(pd[:], lhsT=ones_col[:, :, 0:1], rhs=pt2[:],
                                 start=(kp == 0), stop=(kp == NLB // 2 - 1), perf_mode=DR)

            # Finale. Order matters: pdc frees the "d" bank and the ou copies
            # free the "po" banks that the next lq tile's denominator/PV
            # matmuls need — emit them first so DVE runs them first.
            # Broadcast raw denominators across partitions via PE, then take
            # the reciprocal on all 128 lanes (a [1,512] single-lane
            # reciprocal is ~2.7us and stalls the PE).
            pdc = small.tile([1, LQ], bf16, tag="pdc", bufs=2)
            with nc.allow_low_precision(reason="denom rounded to bf16 as matmul operand"):
                nc.vector.tensor_copy(out=pdc[:], in_=pd[:])
            ou = [oup.tile([P, LQ], bf16, tag="ou", name=f"ou{i}") for i in range(CS)]
            for c_ in range(CS):
                nc.vector.tensor_copy(out=ou[c_][:], in_=po[c_][:])
            residual_pass(lt)
            # On the last tile nothing follows, so use the idle "s" slots and
            # let the finale matmuls/DVE run with full double-buffering.
            fin_ps, fin_tag = (psA, "po") if lt < NLT - 1 else (psS, "s")
            pb = fin_ps.tile([P, LQ], f32, tag=fin_tag, name="ps_b")
            nc.tensor.matmul(pb[:], lhsT=ones_row[:], rhs=pdc[:], start=True, stop=True)
            rb = finp.tile([P, LQ], f32, tag="rb")
            nc.vector.reciprocal(out=rb[:], in_=pb[:])

            for co_s in range(CS):
                pz = fin_ps.tile([P, LQ], f32, tag=fin_tag, name="ps_z")
                for ci in range(CS):
                    nc.tensor.matmul(pz[:], lhsT=wsb["wp"][:, ci, ts(co_s, P)],
                                     rhs=ou[ci][:],
                                     start=(ci == 0), stop=(ci == CS - 1))
                fin = finp.tile([P, LQ], f32, tag="fin")
                nc.vector.tensor_tensor(out=fin[:], in0=pz[:], in1=rb[:], op=ALU.mult)
                nc.vector.tensor_tensor(out=fin[:], in0=fin[:],
                                        in1=xt[:, co_s, ts(lt, LQ)], op=ALU.add)
                nc.sync.dma_start(out=out_dv[:, co_s, ts(lt, LQ)], in_=fin[:])

    nc.compile()
    return nc


def get_nc():
    if "nc" not in _CACHE:
        _CACHE["nc"] = _build_nc()
    return _CACHE["nc"]


def _g0_const():
    g = np.zeros((P, 2), np.float32)
    g[0:CPG, 0] = 1.0 / CPG
    g[CPG:P, 1] = 1.0 / CPG
    return g


def _sel_const():
    s = np.zeros((2, P), np.float32)
    s[0, 0:CPG] = 1.0
    s[1, CPG:P] = 1.0
    return s


def prep_inputs(x, gamma, beta, wq, bq, wk, bk, wv, bv, wp, bp):
    """Host-side layout prep (transposes / reshapes / bf16 weight casts, plus
    folding the 1/sqrt(C) attention scale into wk/bk). Per-core input maps."""
    import ml_dtypes

    f = np.float32
    bf = ml_dtypes.bfloat16
    x = np.asarray(x, f)
    scale = f(C) ** f(-0.5)

    def wprep(w):
        w = np.asarray(w, f)
        return np.ascontiguousarray(w.reshape(CS, P, C).transpose(1, 0, 2)).astype(bf)

    def vprep(v):
        v = np.asarray(v, f)
        return np.ascontiguousarray(v.reshape(CS, P).T)

    shared = {
        "wq": wprep(wq), "wk": wprep(np.asarray(wk, f) * scale),
        "wv": wprep(wv), "wp": wprep(wp),
        "vp": np.ascontiguousarray(np.concatenate(
            [vprep(gamma), vprep(beta), vprep(bq),
             vprep(np.asarray(bk, f) * scale), vprep(bp)], axis=1)),
        "bv_bcast": np.ascontiguousarray(np.broadcast_to(np.asarray(bv, f), (P, C))),
        "g0": _g0_const(), "sel": _sel_const(),
    }
    in_maps = []
    for b in range(N_CORES):
        m = dict(shared)
        xtb = np.ascontiguousarray(x[b].T)                       # [C, L]
        m["xb"] = xtb.astype(bf)
        # [NLT, P, CS, LQ]: per-lq-tile chunks of x^T in [p, s, j] layout
        m["xr"] = np.ascontiguousarray(
            xtb.reshape(CS, P, NLT, LQ).transpose(2, 1, 0, 3))
        in_maps.append(m)
    return in_maps


def run(inputs, trace=False, **kw):
    from concourse.bass_utils import run_bass_kernel_spmd

    nc = get_nc()
    in_maps = prep_inputs(**inputs)
    return run_bass_kernel_spmd(nc, in_maps, core_ids=list(range(N_CORES)),
                                trace=trace, **kw)


def kernel(**inputs) -> np.ndarray:
    res = run(inputs)
    out = np.empty((B, L, C), np.float32)
    for b in range(N_CORES):
        out[b] = res.results[b]["out_t"].T
    return out


# revision 63
# speedup vs baseline: 1.7616x; 1.0559x over previous
"""Trainium2 Bass kernel for nn_AttentionBlock (B=8, L=2048, C=512, GroupNorm(8) +
single-head attention + residual), data-parallel over batch across 8 NeuronCores.

Self-contained: hardcodes shapes/sharding. kernel(**inputs) -> np.ndarray [B,L,C].

Dataflow (per core, one batch element, everything channel-major / "transposed"):
  x^T [C,L] (f32) --bn_stats/group-reduce--> h^T = a_c * x^T + b_c  (f32 + bf16 copy)
  Q^T = wq^T h^T + bq ;  K^T = (wk*scale)^T h^T + bk*scale  (scale folded on host)
  V   = h^T-chunks^T @ wv + bv           (natural [L,C] layout)
  per 512-wide lq tile:
     for each 128-key block: S^T = K^T-chunk^T @ Q^T (PSUM f32); P = exp(S^T) (bf16)
     O^T  += V-chunk^T @ P  (PSUM f32 accum over key blocks), denom += 1^T @ P
     out^T = h^T + (wp^T O^T) * (1/denom) + bp      (f32 combine)
Matmul operands are bf16 (1 cyc/row on PE); accumulation always fp32 in PSUM.
Host side transposes x per batch, casts weights to bf16, transposes output back.
"""

import numpy as np

B, L, C = 8, 2048, 512
GROUPS = 8
EPS = 1e-3
P = 128
CS = C // P            # 4 channel subtiles of 128
LQ = 512               # lq tile width (matmul free dim)
NLT = L // LQ          # 4 lq tiles
NLB = L // P           # 16 key/l blocks
CPG = C // GROUPS      # 64 channels per group
N_CORES = 8

_CACHE = {}


def _build_nc():
    from contextlib import ExitStack

    import concourse.bass as bass
    import concourse.mybir as mybir
    import concourse.tile as tile
    from concourse import bacc
    from concourse.bass import ts

    f32 = mybir.dt.float32
    bf16 = mybir.dt.bfloat16
    fp8 = mybir.dt.float8e4
    DR = mybir.MatmulPerfMode.DoubleRow
    AF = mybir.ActivationFunctionType
    ALU = mybir.AluOpType

    nc = bacc.Bacc(trn_type="TRN2")

    xr_d = nc.dram_tensor("xr", [NLT, P, CS, LQ], f32, kind="ExternalInput")
    xb_d = nc.dram_tensor("xb", [C, L], bf16, kind="ExternalInput")
    w_d = {
        n: nc.dram_tensor(n, [P, CS, C], fp8, kind="ExternalInput")
        for n in ("wq", "wk", "wv", "wp")
    }
    # packed per-channel vectors: [gamma, beta, bq, bk, bp] x CS columns
    vp_d = nc.dram_tensor("vp", [P, 5 * CS], f32, kind="ExternalInput")
    bvb_d = nc.dram_tensor("bv_bcast", [P, C], f32, kind="ExternalInput")
    g0_d = nc.dram_tensor("g0", [P, 2], f32, kind="ExternalInput")
    sel_d = nc.dram_tensor("sel", [2, P], f32, kind="ExternalInput")
    out_d = nc.dram_tensor("out_t", [C, L], f32, kind="ExternalOutput")

    xb_dv = xb_d[:].rearrange("(s p) l -> p s l", p=P)
    out_dv = out_d[:].rearrange("(s p) l -> p s l", p=P)

    with tile.TileContext(nc) as tc, ExitStack() as ctx:
        consts = ctx.enter_context(tc.tile_pool(name="consts", bufs=1))
        data = ctx.enter_context(tc.tile_pool(name="data", bufs=1))
        small = ctx.enter_context(tc.tile_pool(name="small", bufs=1))
        ptp = ctx.enter_context(tc.tile_pool(name="ptp", bufs=3))
        oup = ctx.enter_context(tc.tile_pool(name="oup", bufs=4))
        finp = ctx.enter_context(tc.tile_pool(name="finp", bufs=2))
        psA = ctx.enter_context(tc.tile_pool(name="psA", bufs=4, space="PSUM"))
        psS = ctx.enter_context(tc.tile_pool(name="psS", bufs=3, space="PSUM"))
        psD = ctx.enter_context(tc.tile_pool(name="psD", bufs=1, space="PSUM"))

        # ---- SBUF residents ----
        xt = data.tile([P, CS, L], f32)       # x^T, then h^T (f32, residual)
        xb = data.tile([P, CS, L], bf16)      # x^T bf16, stats fast path
        hb = data.tile([P, CS, L], fp8)       # h^T fp8 (matmul operand)
        qt = data.tile([P, CS, L], fp8)       # Q^T (fp8: attention matmuls run
        kt = data.tile([P, CS, L], fp8)       # DoubleRow, 2x PE throughput)
        vt = data.tile([P, NLB, C], fp8)      # V natural, [l%P, l//P, c]
        wsb = {n: consts.tile([P, CS, C], fp8, name=f"w_{n}") for n in w_d}
        vp = consts.tile([P, 5 * CS], f32)
        GAM, BET, BQ, BK, BP = (vp[:, i * CS:(i + 1) * CS] for i in range(5))
        bvb = consts.tile([P, C], f32)
        g0 = consts.tile([P, 2], f32)
        sel = consts.tile([2, P], f32)
        ones_col = consts.tile([P, 2, 16], fp8)   # [:, :, 0:1] = DoubleRow ones
        ones_row = consts.tile([1, P], bf16)
        eps2 = consts.tile([2, 1], f32)

        # ---- loads + constants ----
        # DMA order IS the critical path: tiny constants first (they gate the
        # stats reduce), then bf16 x (gates bn_stats), then weights (gate the
        # first projection matmuls), then f32 x (residual only — needed late).
        nc.gpsimd.dma_start(out=g0[:], in_=g0_d[:])
        nc.gpsimd.dma_start(out=sel[:], in_=sel_d[:])
        nc.gpsimd.dma_start(out=vp[:], in_=vp_d[:])
        for s in range(CS):
            nc.sync.dma_start(out=xb[:, s, :], in_=xb_dv[:, s, :])
        for n in ("wq", "wk", "wv", "wp"):
            nc.sync.dma_start(out=wsb[n][:], in_=w_d[n][:])
        nc.sync.dma_start(out=bvb[:], in_=bvb_d[:])
        # f32 x only feeds the residual add in the lt-th finale — stream it
        # per lq tile so it never competes with the latency-critical loads.
        for lt in range(NLT):
            nc.sync.dma_start(out=xt[:, :, ts(lt, LQ)], in_=xr_d[lt])
        nc.vector.memset(ones_col[:], 1.0)
        nc.vector.memset(ones_row[:], 1.0)
        nc.vector.memset(eps2[:], EPS)

        # ---- GroupNorm stats ----
        # per-channel (partition) mean/var over L via bn_stats, then group
        # aggregation across partitions with a tiny fp32 matmul.
        st = small.tile([P, CS, 2], f32)      # (mean_c, E[x^2]_c) per subtile
        # subtiles 0..2 on DVE (bn_stats); subtile 3 on the otherwise-idle ACT
        # via activation accum_out (sum and sum-of-squares along L).
        for s in range(CS - 1):
            st6 = small.tile([P, 4, 6], f32, tag="st6", bufs=2)
            for j in range(4):
                nc.vector.bn_stats(out=st6[:, j, :], in_=xb[:, s, ts(j, 512)])
            mv = small.tile([P, 2], f32, tag="mv", bufs=2)
            nc.vector.bn_aggr(out=mv[:], in_=st6[:])
            nc.vector.tensor_copy(out=st[:, s, 0:1], in_=mv[:, 0:1])
            nc.vector.tensor_tensor(out=st[:, s, 1:2], in0=mv[:, 0:1], in1=mv[:, 0:1], op=ALU.mult)
            nc.vector.tensor_tensor(out=st[:, s, 1:2], in0=st[:, s, 1:2], in1=mv[:, 1:2], op=ALU.add)
        s3 = CS - 1
        ssum = small.tile([P, 2], f32)        # (sum, sumsq) of subtile 3
        gscr = small.tile([P, L], bf16)
        nc.scalar.activation(out=gscr[:], in_=xb[:, s3, :], func=AF.Identity,
                             accum_out=ssum[:, 0:1])
        nc.scalar.activation(out=gscr[:], in_=xb[:, s3, :], func=AF.Square,
                             accum_out=ssum[:, 1:2])
        nc.vector.tensor_scalar(out=st[:, s3, :], in0=ssum[:], scalar1=1.0 / L, scalar2=None,
                                op0=ALU.mult)

        psg = psD.tile([2, 2 * CS], f32, tag="d")   # [group-half, (s, stat)]
        nc.tensor.matmul(psg[:], lhsT=g0[:], rhs=st[:].rearrange("p a b -> p (a b)"),
                         start=True, stop=True)
        pst = small.tile([2, 2 * CS], f32)
        nc.vector.tensor_copy(out=pst[:], in_=psg[:])
        pstv = pst[:].rearrange("p (s k) -> p s k", k=2)
        msq = small.tile([2, CS], f32)
        nc.vector.tensor_tensor(out=msq[:], in0=pstv[:, :, 0], in1=pstv[:, :, 0], op=ALU.mult)
        grp = small.tile([2, 2 * CS], f32)     # [:, :CS]=rstd_g, [:, CS:]=mean_g
        nc.vector.tensor_tensor(out=grp[:, 0:CS], in0=pstv[:, :, 1], in1=msq[:], op=ALU.subtract)
        nc.scalar.activation(out=grp[:, 0:CS], in_=grp[:, 0:CS], func=AF.Sqrt,
                             bias=eps2[:], scale=1.0)
        nc.vector.reciprocal(out=grp[:, 0:CS], in_=grp[:, 0:CS])
        nc.vector.tensor_copy(out=grp[:, CS:], in_=pstv[:, :, 0])
        # dummy Exp: pulls the Exp table-set load (~2.7us) off the first real
        # exp's critical path; Identity (used by the Q/K copies) is a filler
        # function present in every set.
        nc.scalar.activation(out=msq[:, 0:1], in_=eps2[:], func=AF.Exp)

        psbc = psD.tile([P, 2 * CS], f32, tag="d")  # broadcast groups -> channels
        nc.tensor.matmul(psbc[:], lhsT=sel[:], rhs=grp[:], start=True, stop=True)
        ab = small.tile([P, 2 * CS], f32)      # [:, :CS]=a_c, [:, CS:]=b_c
        nc.vector.tensor_tensor(out=ab[:, 0:CS], in0=GAM, in1=psbc[:, 0:CS], op=ALU.mult)
        nc.vector.tensor_tensor(out=ab[:, CS:], in0=psbc[:, CS:], in1=ab[:, 0:CS], op=ALU.mult)
        nc.vector.tensor_tensor(out=ab[:, CS:], in0=BET, in1=ab[:, CS:], op=ALU.subtract)
        # residual pass scalars with the output-projection bias folded in:
        # out = (a*x + b + bp) + Z/denom
        ab2 = small.tile([P, CS], f32)
        nc.vector.tensor_tensor(out=ab2[:], in0=ab[:, CS:], in1=BP, op=ALU.add)

        # ---- normalize: h^T = a*x^T + b ----
        # bf16 copy first (it gates all matmuls), split across DVE and ACT so
        # all four subtiles are ready ~2x sooner; the f32 in-place pass only
        # feeds the residual add much later, so it runs off the critical path.
        for s in range(CS):
            if s < 2:
                nc.vector.tensor_scalar(out=hb[:, s, :], in0=xb[:, s, :],
                                        scalar1=ab[:, s:s + 1], scalar2=ab[:, CS + s:CS + s + 1],
                                        op0=ALU.mult, op1=ALU.add)
            else:
                nc.scalar.activation(out=hb[:, s, :], in_=xb[:, s, :], func=AF.Identity,
                                     bias=ab[:, CS + s:CS + s + 1], scale=ab[:, s:s + 1])
        def residual_pass(lt):
            # h^T + bp for the lt-th finale, in place over the streamed f32 x
            for s in range(CS):
                nc.vector.tensor_scalar(out=xt[:, s, ts(lt, LQ)], in0=xt[:, s, ts(lt, LQ)],
                                        scalar1=ab[:, s:s + 1], scalar2=ab2[:, s:s + 1],
                                        op0=ALU.mult, op1=ALU.add)

        # ---- projections ----
        def project_t(w, bias, dst):
            # dst[:, co_s, l] = sum_ci w[ci, co]^T h^T + bias[co]; weights come
            # in x8 (fp8 range), the copy rescales by 1/8.
            for co_s in range(CS):
                for lt in range(NLT):
                    ps = psS.tile([P, LQ], f32, tag="s", name="ps_prj")
                    for cp in range(2):
                        nc.tensor.matmul(ps[:], lhsT=w[:, 2 * cp:2 * cp + 2, ts(co_s, P)],
                                         rhs=hb[:, 2 * cp:2 * cp + 2, ts(lt, LQ)],
                                         start=(cp == 0), stop=(cp == 1), perf_mode=DR)
                    nc.scalar.activation(out=dst[:, co_s, ts(lt, LQ)], in_=ps[:],
                                         func=AF.Identity, bias=bias[:, co_s:co_s + 1],
                                         scale=1.0 / 8)

        project_t(wsb["wq"], BQ, qt)
        project_t(wsb["wk"], BK, kt)

        for lb in range(NLB):
            ps = psS.tile([P, C], f32, tag="s", name="ps_v")
            for cp in range(2):
                nc.tensor.matmul(ps[:], lhsT=hb[:, 2 * cp:2 * cp + 2, ts(lb, P)],
                                 rhs=wsb["wv"][:, 2 * cp:2 * cp + 2, :],
                                 start=(cp == 0), stop=(cp == 1), perf_mode=DR)
            # V stays scaled x4 (wv, bv x4 on host); the 4x8=32 factor from
            # V and wp is divided out of the softmax denominators below.
            nc.vector.tensor_add(out=vt[:, lb, :], in0=ps[:], in1=bvb[:])

        # ---- attention + output projection, per lq tile ----
        for lt in range(NLT):
            po = [psA.tile([P, LQ], f32, tag="po", name=f"po{i}") for i in range(CS)]
            pd = psD.tile([1, LQ], f32, tag="d", name="pd")
            for kp in range(NLB // 2):
                # S^T for a pair of key blocks: 2 DoubleRow matmuls each
                # (contraction 256 = two channel subtiles per matmul)
                pt2 = ptp.tile([P, 2, LQ], fp8, tag="pt")
                for i in range(2):
                    kb = 2 * kp + i
                    ps = psS.tile([P, LQ], f32, tag="s", name="ps_s")
                    for cp in range(2):
                        nc.tensor.matmul(ps[:], lhsT=kt[:, 2 * cp:2 * cp + 2, ts(kb, P)],
                                         rhs=qt[:, 2 * cp:2 * cp + 2, ts(lt, LQ)],
                                         start=(cp == 0), stop=(cp == 1), perf_mode=DR)
                    nc.scalar.activation(out=pt2[:, i, :], in_=ps[:], func=AF.Exp)
                # PV + denominator over the key-block pair, DoubleRow again
                for c_ in range(CS):
                    nc.tensor.matmul(po[c_][:], lhsT=vt[:, 2 * kp:2 * kp + 2, ts(c_, P)],
                                     rhs=pt2[:], start=(kp == 0), stop=(kp == NLB // 2 - 1),
                                     perf_mode=DR)
                nc.tensor.matmul(pd[:], lhsT=ones_col[:, :, 0:1], rhs=pt2[:],
                                 start=(kp == 0), stop=(kp == NLB // 2 - 1), perf_mode=DR)

            # Finale. Order matters: pdc frees the "d" bank and the ou copies
            # free the "po" banks that the next lq tile's denominator/PV
            # matmuls need — emit them first so DVE runs them first.
            # Broadcast raw denominators across partitions via PE, then take
            # the reciprocal on all 128 lanes (a [1,512] single-lane
            # reciprocal is ~2.7us and stalls the PE).
            pdc = small.tile([1, LQ], bf16, tag="pdc", bufs=2)
            with nc.allow_low_precision(reason="denom rounded to bf16 as matmul operand"):
                nc.vector.tensor_scalar(out=pdc[:], in0=pd[:], scalar1=8.0, scalar2=None,
                                        op0=ALU.mult)
            ou = oup.tile([P, CS, LQ], fp8, tag="ou")
            for c_ in range(CS):
                nc.vector.tensor_copy(out=ou[:, c_, :], in_=po[c_][:])
            residual_pass(lt)
            # On the last tile nothing follows, so use the idle "s" slots and
            # let the finale matmuls/DVE run with full double-buffering.
            fin_ps, fin_tag = (psA, "po") if lt < NLT - 1 else (psS, "s")
            pb = fin_ps.tile([P, LQ], f32, tag=fin_tag, name="ps_b")
            nc.tensor.matmul(pb[:], lhsT=ones_row[:], rhs=pdc[:], start=True, stop=True)
            rb = finp.tile([P, LQ], f32, tag="rb")
            nc.vector.reciprocal(out=rb[:], in_=pb[:])

            for co_s in range(CS):
                pz = fin_ps.tile([P, LQ], f32, tag=fin_tag, name="ps_z")
                for cp in range(2):
                    nc.tensor.matmul(pz[:], lhsT=wsb["wp"][:, 2 * cp:2 * cp + 2, ts(co_s, P)],
                                     rhs=ou[:, 2 * cp:2 * cp + 2, :],
                                     start=(cp == 0), stop=(cp == 1), perf_mode=DR)
                fin = finp.tile([P, LQ], f32, tag="fin")
                nc.vector.tensor_tensor(out=fin[:], in0=pz[:], in1=rb[:], op=ALU.mult)
                nc.vector.tensor_tensor(out=fin[:], in0=fin[:],
                                        in1=xt[:, co_s, ts(lt, LQ)], op=ALU.add)
                nc.sync.dma_start(out=out_dv[:, co_s, ts(lt, LQ)], in_=fin[:])

    nc.compile()
    return nc


def get_nc():
    if "nc" not in _CACHE:
        _CACHE["nc"] = _build_nc()
    return _CACHE["nc"]


def _g0_const():
    g = np.zeros((P, 2), np.float32)
    g[0:CPG, 0] = 1.0 / CPG
    g[CPG:P, 1] = 1.0 / CPG
    return g


def _sel_const():
    s = np.zeros((2, P), np.float32)
    s[0, 0:CPG] = 1.0
    s[1, CPG:P] = 1.0
    return s


def prep_inputs(x, gamma, beta, wq, bq, wk, bk, wv, bv, wp, bp):
    """Host-side layout prep (transposes / reshapes / bf16 weight casts, plus
    folding the 1/sqrt(C) attention scale into wk/bk). Per-core input maps."""
    import ml_dtypes

    f = np.float32
    bf = ml_dtypes.bfloat16
    f8 = ml_dtypes.float8_e4m3fn
    x = np.asarray(x, f)
    scale = f(C) ** f(-0.5)

    def wprep(w, s):
        # x8 / x4 pre-scale keeps the ~N(0, 0.02) weights in fp8e4m3's normal
        # range; the kernel divides the factors back out (copy scale=1/8 for
        # q/k, 4*8=32 folded into the softmax denominators for v/p).
        w = np.asarray(w, f) * s
        return np.ascontiguousarray(w.reshape(CS, P, C).transpose(1, 0, 2)).astype(f8)

    def vprep(v):
        v = np.asarray(v, f)
        return np.ascontiguousarray(v.reshape(CS, P).T)

    shared = {
        "wq": wprep(wq, 8), "wk": wprep(np.asarray(wk, f) * scale, 8),
        "wv": wprep(wv, 1), "wp": wprep(wp, 8),
        "vp": np.ascontiguousarray(np.concatenate(
            [vprep(gamma), vprep(beta), vprep(bq),
             vprep(np.asarray(bk, f) * scale), vprep(bp)], axis=1)),
        "bv_bcast": np.ascontiguousarray(
            np.broadcast_to(np.asarray(bv, f), (P, C))),
        "g0": _g0_const(), "sel": _sel_const(),
    }
    in_maps = []
    for b in range(N_CORES):
        m = dict(shared)
        xtb = np.ascontiguousarray(x[b].T)                       # [C, L]
        m["xb"] = xtb.astype(bf)
        # [NLT, P, CS, LQ]: per-lq-tile chunks of x^T in [p, s, j] layout
        m["xr"] = np.ascontiguousarray(
            xtb.reshape(CS, P, NLT, LQ).transpose(2, 1, 0, 3))
        in_maps.append(m)
    return in_maps


def run(inputs, trace=False, **kw):
    from concourse.bass_utils import run_bass_kernel_spmd

    nc = get_nc()
    in_maps = prep_inputs(**inputs)
    return run_bass_kernel_spmd(nc, in_maps, core_ids=list(range(N_CORES)),
                                trace=trace, **kw)


def kernel(**inputs) -> np.ndarray:
    res = run(inputs)
    out = np.empty((B, L, C), np.float32)
    for b in range(N_CORES):
        out[b] = res.results[b]["out_t"].T
    return out
